# revision 4
# baseline (speedup 1.0000x reference)
"""CRF negative-log-likelihood (mean) on 8 Trainium2 NeuronCores.

Strategy (data-parallel over batch, 64 sequences/core):

Denominator — forward algorithm in the multiplicative domain with a constant
per-step shift c (no per-step normalization; fp32 range is sufficient):
    P_0 = exp(em_0 - c) * exp(start + c)            [T=128, B_loc=64]
    P_i = (E^T P_{i-1}) o exp(em_i - c),  E = exp(transitions)
    den_b = (S-1)*c + ln( sum_t P_{S-1}[t,b] * exp(end[t]) )
Per step: one 128x128 @ 128x64 matmul (E stationary on the PE) and one DVE
tensor_tensor multiply out of PSUM with the precomputed exp(em - c) slice.
Emissions are host-permuted to [T, S, B_loc] so the chain needs no on-device
transposes; exp(em - c) is computed in bulk on the ACT engine off the
critical path.

Numerator — only its batch-sum is needed for the mean, so all gathers
(emissions at gold tags, transition scores, start/end) are indirect-DMA
element gathers followed by reductions.

Each core emits [sum_b ln T_b, numerator_sum]; the host combines:
    loss = sum_cores(out0 - out1) / B + (S-1)*c
"""

from contextlib import ExitStack

import numpy as np

import concourse.bass as bass
import concourse.bacc as bacc
import concourse.mybir as mybir
import concourse.tile as tile
from concourse.bass_utils import run_bass_kernel_spmd

F32 = mybir.dt.float32
I32 = mybir.dt.int32
AF = mybir.ActivationFunctionType
ALU = mybir.AluOpType
AX = mybir.AxisListType

B, S, T = 512, 512, 128
N_CORES = 8
BL = B // N_CORES
C_SHIFT = float(np.float32(np.log(128.0) + 0.5))


def _build_nc(chunk=32):
    n_chunks = S // chunk
    nc = bacc.Bacc("TRN2", target_bir_lowering=False, debug=False)

    emt = nc.declare_dram_parameter("emt", [T, S, BL], F32, isOutput=False)
    tags_d = nc.declare_dram_parameter("tags", [BL, S], I32, isOutput=False)
    trans_d = nc.declare_dram_parameter("trans", [T, T], F32, isOutput=False)
    start_d = nc.declare_dram_parameter("startv", [T], F32, isOutput=False)
    end_d = nc.declare_dram_parameter("endv", [T], F32, isOutput=False)
    out_d = nc.declare_dram_parameter("out", [2], F32, isOutput=True)

    with ExitStack() as ctx:
        tc = ctx.enter_context(tile.TileContext(nc))
        constp = ctx.enter_context(tc.tile_pool(name="const", bufs=1))
        emp = ctx.enter_context(tc.tile_pool(name="em", bufs=2))
        wp = ctx.enter_context(tc.tile_pool(name="w", bufs=1))
        statep = ctx.enter_context(tc.tile_pool(name="state", bufs=2))
        psump = ctx.enter_context(tc.tile_pool(name="psum", bufs=4, space="PSUM"))
        psumm = ctx.enter_context(tc.tile_pool(name="psumm", bufs=1, space="PSUM"))
        nump = ctx.enter_context(tc.tile_pool(name="num", bufs=1))

        # ---- constants ----
        posc_sb = constp.tile([T, 1], F32)
        nc.vector.memset(posc_sb[:], C_SHIFT)
        negc_sb = constp.tile([T, 1], F32)
        nc.vector.memset(negc_sb[:], -C_SHIFT)

        trans_sb = constp.tile([T, T], F32)
        nc.sync.dma_start(trans_sb[:], trans_d[:])
        E_sb = constp.tile([T, T], F32)
        nc.scalar.activation(E_sb[:], trans_sb[:], AF.Exp)

        start_sb = constp.tile([T, 1], F32)
        nc.sync.dma_start(start_sb[:], start_d[:].rearrange("(t o) -> t o", o=1))
        startc_sb = constp.tile([T, 1], F32)  # exp(start + c)
        nc.scalar.activation(startc_sb[:], start_sb[:], AF.Exp, bias=posc_sb[:, 0:1])

        end_sb = constp.tile([T, 1], F32)
        nc.sync.dma_start(end_sb[:], end_d[:].rearrange("(t o) -> t o", o=1))
        endexp_sb = constp.tile([T, 1], F32)  # exp(end)
        nc.scalar.activation(endexp_sb[:], end_sb[:], AF.Exp)

        ones_sb = constp.tile([T, 1], F32)
        nc.vector.memset(ones_sb[:], 1.0)

        # ---- W chunks: W[:, s*BL + b] = exp(em[t, s, b] - c) ----
        w_tiles = []
        for k in range(n_chunks):
            em_t = emp.tile([T, chunk * BL], F32, tag="emchunk")
            nc.sync.dma_start(
                em_t[:],
                emt[:, k * chunk:(k + 1) * chunk, :].rearrange("t s b -> t (s b)"),
            )
            w_t = wp.tile([T, chunk * BL], F32, tag=f"w{k}")
            nc.scalar.activation(w_t[:], em_t[:], AF.Exp, bias=negc_sb[:, 0:1])
            w_tiles.append(w_t)

        def w_slice(i):
            return w_tiles[i // chunk][:, (i % chunk) * BL:(i % chunk) * BL + BL]

        # ---- chain ----
        state = statep.tile([T, BL], F32, tag="state")
        nc.vector.tensor_scalar(state[:], w_slice(0), startc_sb[:, 0:1], None, ALU.mult)
        for i in range(1, S):
            q = psump.tile([T, BL], F32, tag="q")
            nc.tensor.matmul(q[:], lhsT=E_sb[:], rhs=state[:], start=True, stop=True)
            new_state = statep.tile([T, BL], F32, tag="state")
            nc.vector.tensor_tensor(new_state[:], q[:], w_slice(i), op=ALU.mult)
            state = new_state

        # ---- denominator reduce: T_b = sum_t state * exp(end) ----
        pf = nump.tile([T, BL], F32)
        nc.vector.tensor_scalar(pf[:], state[:], endexp_sb[:, 0:1], None, ALU.mult)
        colsum = psumm.tile([1, BL], F32, tag="colsum")
        nc.tensor.matmul(colsum[:], lhsT=ones_sb[:], rhs=pf[:], start=True, stop=True)
        den_ln = nump.tile([1, BL], F32)
        nc.scalar.activation(den_ln[:], colsum[:], AF.Ln)
        den_sum = nump.tile([1, 1], F32)
        nc.vector.tensor_reduce(den_sum[:], den_ln[:], axis=AX.X, op=ALU.add)

        # ---- numerator ----
        tags_sb = nump.tile([BL, S], I32)
        nc.sync.dma_start(tags_sb[:], tags_d[:])
        tags_f = nump.tile([BL, S], F32)
        nc.vector.tensor_copy(tags_f[:], tags_sb[:])

        # offs_em[b, s] = tags*(S*BL) + s*BL + b   (emt flat index)
        sb_base = nump.tile([BL, S], I32)
        nc.gpsimd.iota(sb_base[:], pattern=[[BL, S]], base=0, channel_multiplier=1)
        sb_base_f = nump.tile([BL, S], F32)
        nc.vector.tensor_copy(sb_base_f[:], sb_base[:])
        offs_em_f = nump.tile([BL, S], F32)
        nc.vector.scalar_tensor_tensor(
            offs_em_f[:], tags_f[:], float(S * BL), sb_base_f[:],
            op0=ALU.mult, op1=ALU.add,
        )
        offs_em = nump.tile([BL, S], I32)
        nc.vector.tensor_copy(offs_em[:], offs_em_f[:])

        # offs_tr[b, s] = tags[b, s]*T + tags[b, s+1]
        offs_tr_f = nump.tile([BL, S - 1], F32)
        nc.vector.scalar_tensor_tensor(
            offs_tr_f[:], tags_f[:, 0:S - 1], float(T), tags_f[:, 1:S],
            op0=ALU.mult, op1=ALU.add,
        )
        offs_tr = nump.tile([BL, S - 1], I32)
        nc.vector.tensor_copy(offs_tr[:], offs_tr_f[:])

        # gathers
        emv = nump.tile([BL, S], F32)
        nc.gpsimd.indirect_dma_start(
            out=emv[:], out_offset=None,
            in_=emt[:].rearrange("t s b -> (t s b)").rearrange("(x o) -> x o", o=1),
            in_offset=bass.IndirectOffsetOnAxis(ap=offs_em[:], axis=0),
        )
        trv = nump.tile([BL, S - 1], F32)
        nc.gpsimd.indirect_dma_start(
            out=trv[:], out_offset=None,
            in_=trans_d[:].rearrange("u v -> (u v)").rearrange("(x o) -> x o", o=1),
            in_offset=bass.IndirectOffsetOnAxis(ap=offs_tr[:], axis=0),
        )
        stv = nump.tile([BL, 1], F32)
        nc.gpsimd.indirect_dma_start(
            out=stv[:], out_offset=None,
            in_=start_d[:].rearrange("(t o) -> t o", o=1),
            in_offset=bass.IndirectOffsetOnAxis(ap=tags_sb[:, 0:1], axis=0),
        )
        env = nump.tile([BL, 1], F32)
        nc.gpsimd.indirect_dma_start(
            out=env[:], out_offset=None,
            in_=end_d[:].rearrange("(t o) -> t o", o=1),
            in_offset=bass.IndirectOffsetOnAxis(ap=tags_sb[:, S - 1:S], axis=0),
        )

        # reduce numerator to [BL, 1] then to scalar
        em_rs = nump.tile([BL, 1], F32)
        nc.vector.tensor_reduce(em_rs[:], emv[:], axis=AX.X, op=ALU.add)
        tr_rs = nump.tile([BL, 1], F32)
        nc.vector.tensor_reduce(tr_rs[:], trv[:], axis=AX.X, op=ALU.add)
        nsum = nump.tile([BL, 1], F32)
        nc.vector.tensor_tensor(nsum[:], em_rs[:], tr_rs[:], op=ALU.add)
        nc.vector.tensor_tensor(nsum[:], nsum[:], stv[:], op=ALU.add)
        nc.vector.tensor_tensor(nsum[:], nsum[:], env[:], op=ALU.add)

        ones64 = nump.tile([BL, 1], F32)
        nc.vector.memset(ones64[:], 1.0)
        numsum_ps = psumm.tile([1, 1], F32, tag="numsum")
        nc.tensor.matmul(numsum_ps[:], lhsT=ones64[:], rhs=nsum[:],
                         start=True, stop=True)

        # ---- assemble output ----
        out_sb = nump.tile([1, 2], F32)
        nc.vector.tensor_copy(out_sb[:, 0:1], den_sum[:])
        nc.vector.tensor_copy(out_sb[:, 1:2], numsum_ps[:])
        nc.sync.dma_start(out_d[:].rearrange("(o x) -> o x", o=1), out_sb[:])

    return nc


_NC_CACHE = {}


def _get_nc():
    if "nc" not in _NC_CACHE:
        nc = _build_nc()
        nc.finalize()
        _NC_CACHE["nc"] = nc
    return _NC_CACHE["nc"]


def kernel(emissions, start_transitions, end_transitions, transitions, tags, mask,
           _trace=False):
    emissions = np.ascontiguousarray(np.asarray(emissions, dtype=np.float32))
    start_transitions = np.ascontiguousarray(
        np.asarray(start_transitions, dtype=np.float32))
    end_transitions = np.ascontiguousarray(
        np.asarray(end_transitions, dtype=np.float32))
    transitions = np.ascontiguousarray(np.asarray(transitions, dtype=np.float32))
    tags = np.ascontiguousarray(np.asarray(tags, dtype=np.int32))
    mask = np.asarray(mask)
    assert emissions.shape == (B, S, T) and tags.shape == (B, S)
    # setup_inputs() produces an all-ones mask; this kernel relies on it.
    assert np.all(mask == 1), "kernel assumes a full (all-ones) mask"

    in_maps = []
    for core in range(N_CORES):
        lo = core * BL
        emt = np.ascontiguousarray(
            np.transpose(emissions[lo:lo + BL], (2, 1, 0)))  # [T, S, BL]
        in_maps.append({
            "emt": emt,
            "tags": np.ascontiguousarray(tags[lo:lo + BL]),
            "trans": transitions,
            "startv": start_transitions,
            "endv": end_transitions,
        })

    nc = _get_nc()
    res = run_bass_kernel_spmd(nc, in_maps, list(range(N_CORES)), trace=_trace)

    total = 0.0
    for r in res.results:
        o = r["out"]
        total += float(o[0]) - float(o[1])
    loss = np.float32(total / B + (S - 1) * C_SHIFT)
    if _trace:
        return loss, res
    return loss


# revision 5
# speedup vs baseline: 3.2698x; 3.2698x over previous
"""CRF negative-log-likelihood (mean) on 8 Trainium2 NeuronCores.

Strategy (data-parallel over batch, 64 sequences/core):

Denominator — forward algorithm in the multiplicative domain with a constant
per-step shift c (no per-step normalization; fp32 range is sufficient):
    P_0 = exp(em_0 - c) * exp(start + c)            [T=128, B_loc=64]
    P_i = (E^T P_{i-1}) o exp(em_i - c),  E = exp(transitions)
    den_b = (S-1)*c + ln( sum_t P_{S-1}[t,b] * exp(end[t]) )
Per step: one 128x128 @ 128x64 matmul (E stationary on the PE) and one DVE
tensor_tensor multiply out of PSUM with the precomputed exp(em - c) slice.
Emissions are host-permuted to [T, S, B_loc] so the chain needs no on-device
transposes; exp(em - c) is computed in bulk on the ACT engine off the
critical path.

Numerator — only its batch-sum is needed for the mean, so all gathers
(emissions at gold tags, transition scores, start/end) are indirect-DMA
element gathers followed by reductions.

Each core emits [sum_b ln T_b, numerator_sum]; the host combines:
    loss = sum_cores(out0 - out1) / B + (S-1)*c
"""

from contextlib import ExitStack

import numpy as np

import concourse.bass as bass
import concourse.bacc as bacc
import concourse.mybir as mybir
import concourse.tile as tile
from concourse.bass_utils import run_bass_kernel_spmd

F32 = mybir.dt.float32
BF16 = mybir.dt.bfloat16
I32 = mybir.dt.int32
AF = mybir.ActivationFunctionType
ALU = mybir.AluOpType
AX = mybir.AxisListType

B, S, T = 512, 512, 128
N_CORES = 8
BL = B // N_CORES
C_SHIFT = float(np.float32(np.log(128.0) + 0.5))


def _build_nc(chunk=32, w_dtype=BF16, state_dtype=BF16):
    n_chunks = S // chunk
    MID = S // 2
    nc = bacc.Bacc("TRN2", target_bir_lowering=False, debug=False)

    emt = nc.declare_dram_parameter("emt", [T, S, BL], F32, isOutput=False)
    tags_d = nc.declare_dram_parameter("tags", [BL, S], I32, isOutput=False)
    trans_d = nc.declare_dram_parameter("trans", [T, T], F32, isOutput=False)
    transT_d = nc.declare_dram_parameter("transT", [T, T], F32, isOutput=False)
    start_d = nc.declare_dram_parameter("startv", [T], F32, isOutput=False)
    end_d = nc.declare_dram_parameter("endv", [T], F32, isOutput=False)
    out_d = nc.declare_dram_parameter("out", [2], F32, isOutput=True)

    with ExitStack() as ctx:
        tc = ctx.enter_context(tile.TileContext(nc))
        constp = ctx.enter_context(tc.tile_pool(name="const", bufs=1))
        emp = ctx.enter_context(tc.tile_pool(name="em", bufs=2))
        wp = ctx.enter_context(tc.tile_pool(name="w", bufs=1))
        statep = ctx.enter_context(tc.tile_pool(name="state", bufs=2))
        stateq = ctx.enter_context(tc.tile_pool(name="stateb", bufs=2))
        psump = ctx.enter_context(tc.tile_pool(name="psum", bufs=3, space="PSUM"))
        psumb = ctx.enter_context(tc.tile_pool(name="psumb", bufs=3, space="PSUM"))
        psumm = ctx.enter_context(tc.tile_pool(name="psumm", bufs=1, space="PSUM"))
        nump = ctx.enter_context(tc.tile_pool(name="num", bufs=1))

        # ---- constants ----
        posc_sb = constp.tile([T, 1], F32)
        nc.vector.memset(posc_sb[:], C_SHIFT)
        negc_sb = constp.tile([T, 1], F32)
        nc.vector.memset(negc_sb[:], -C_SHIFT)

        trans_sb = constp.tile([T, T], F32)
        nc.sync.dma_start(trans_sb[:], trans_d[:])
        E_sb = constp.tile([T, T], state_dtype)
        nc.scalar.activation(E_sb[:], trans_sb[:], AF.Exp)

        transT_sb = constp.tile([T, T], F32)
        nc.sync.dma_start(transT_sb[:], transT_d[:])
        ET_sb = constp.tile([T, T], state_dtype)
        nc.scalar.activation(ET_sb[:], transT_sb[:], AF.Exp)

        start_sb = constp.tile([T, 1], F32)
        nc.sync.dma_start(start_sb[:], start_d[:].rearrange("(t o) -> t o", o=1))
        startc_sb = constp.tile([T, 1], F32)
        nc.scalar.activation(startc_sb[:], start_sb[:], AF.Exp, bias=posc_sb[:, 0:1])

        end_sb = constp.tile([T, 1], F32)
        nc.sync.dma_start(end_sb[:], end_d[:].rearrange("(t o) -> t o", o=1))
        endexp_sb = constp.tile([T, 1], F32)
        nc.scalar.activation(endexp_sb[:], end_sb[:], AF.Exp)

        ones_sb = constp.tile([T, 1], F32)
        nc.vector.memset(ones_sb[:], 1.0)

        # ---- W chunks ----
        w_tiles = [None] * n_chunks
        order = []
        lo_i, hi_i = 0, n_chunks - 1
        while lo_i <= hi_i:
            order.append(lo_i)
            if hi_i != lo_i:
                order.append(hi_i)
            lo_i += 1
            hi_i -= 1
        for k in order:
            em_t = emp.tile([T, chunk * BL], F32, tag="emchunk")
            nc.sync.dma_start(
                em_t[:],
                emt[:, k * chunk:(k + 1) * chunk, :].rearrange("t s b -> t (s b)"),
            )
            w_t = wp.tile([T, chunk * BL], w_dtype, tag=f"w{k}")
            nc.scalar.activation(w_t[:], em_t[:], AF.Exp, bias=negc_sb[:, 0:1])
            w_tiles[k] = w_t

        def w_slice(i):
            return w_tiles[i // chunk][:, (i % chunk) * BL:(i % chunk) * BL + BL]

        # ---- chain states ----
        fstate = statep.tile([T, BL], state_dtype, tag="fstate")
        nc.vector.tensor_scalar(fstate[:], w_slice(0), startc_sb[:, 0:1], None,
                                ALU.mult)
        bstate = stateq.tile([T, BL], state_dtype, tag="bstate")
        nc.vector.tensor_scalar(bstate[:], w_slice(S - 1), endexp_sb[:, 0:1], None,
                                ALU.mult)

        fi = 1          # next fwd step: P_fi        (up to MID)
        bi = S - 2      # next bwd step: A_bi        (down to MID+1)
        while fi <= MID or bi >= MID + 1:
            if fi <= MID:
                q = psump.tile([T, BL], F32, tag="q")
                nc.tensor.matmul(q[:], lhsT=E_sb[:], rhs=fstate[:],
                                 start=True, stop=True)
                nf = statep.tile([T, BL], state_dtype, tag="fstate")
                nc.vector.tensor_tensor(nf[:], q[:], w_slice(fi), op=ALU.mult)
                fstate = nf
                fi += 1
            if bi >= MID + 1:
                qb = psumb.tile([T, BL], F32, tag="qb")
                nc.tensor.matmul(qb[:], lhsT=ET_sb[:], rhs=bstate[:],
                                 start=True, stop=True)
                nb = stateq.tile([T, BL], state_dtype, tag="bstate")
                nc.vector.tensor_tensor(nb[:], qb[:], w_slice(bi), op=ALU.mult)
                bstate = nb
                bi -= 1

        # join: Bt_MID = E @ A_{MID+1}; T_b = sum_t P_MID o Bt_MID
        qb = psumb.tile([T, BL], F32, tag="qb")
        nc.tensor.matmul(qb[:], lhsT=ET_sb[:], rhs=bstate[:], start=True, stop=True)
        pf = nump.tile([T, BL], F32)
        nc.vector.tensor_tensor(pf[:], qb[:], fstate[:], op=ALU.mult)
        colsum = psumm.tile([1, BL], F32, tag="colsum")
        nc.tensor.matmul(colsum[:], lhsT=ones_sb[:], rhs=pf[:], start=True, stop=True)
        den_ln = nump.tile([1, BL], F32)
        nc.scalar.activation(den_ln[:], colsum[:], AF.Ln)
        den_sum = nump.tile([1, 1], F32)
        nc.vector.tensor_reduce(den_sum[:], den_ln[:], axis=AX.X, op=ALU.add)

        # ---- numerator ----
        tags_sb = nump.tile([BL, S], I32)
        nc.sync.dma_start(tags_sb[:], tags_d[:])
        tags_f = nump.tile([BL, S], F32)
        nc.vector.tensor_copy(tags_f[:], tags_sb[:])

        sb_base = nump.tile([BL, S], I32)
        nc.gpsimd.iota(sb_base[:], pattern=[[BL, S]], base=0, channel_multiplier=1)
        sb_base_f = nump.tile([BL, S], F32)
        nc.vector.tensor_copy(sb_base_f[:], sb_base[:])
        offs_em_f = nump.tile([BL, S], F32)
        nc.vector.scalar_tensor_tensor(
            offs_em_f[:], tags_f[:], float(S * BL), sb_base_f[:],
            op0=ALU.mult, op1=ALU.add,
        )
        offs_em = nump.tile([BL, S], I32)
        nc.vector.tensor_copy(offs_em[:], offs_em_f[:])

        offs_tr_f = nump.tile([BL, S - 1], F32)
        nc.vector.scalar_tensor_tensor(
            offs_tr_f[:], tags_f[:, 0:S - 1], float(T), tags_f[:, 1:S],
            op0=ALU.mult, op1=ALU.add,
        )
        offs_tr = nump.tile([BL, S - 1], I32)
        nc.vector.tensor_copy(offs_tr[:], offs_tr_f[:])

        emv = nump.tile([BL, S], F32)
        nc.gpsimd.indirect_dma_start(
            out=emv[:], out_offset=None,
            in_=emt[:].rearrange("t s b -> (t s b)").rearrange("(x o) -> x o", o=1),
            in_offset=bass.IndirectOffsetOnAxis(ap=offs_em[:], axis=0),
        )
        trv = nump.tile([BL, S - 1], F32)
        nc.gpsimd.indirect_dma_start(
            out=trv[:], out_offset=None,
            in_=trans_d[:].rearrange("u v -> (u v)").rearrange("(x o) -> x o", o=1),
            in_offset=bass.IndirectOffsetOnAxis(ap=offs_tr[:], axis=0),
        )
        stv = nump.tile([BL, 1], F32)
        nc.gpsimd.indirect_dma_start(
            out=stv[:], out_offset=None,
            in_=start_d[:].rearrange("(t o) -> t o", o=1),
            in_offset=bass.IndirectOffsetOnAxis(ap=tags_sb[:, 0:1], axis=0),
        )
        env = nump.tile([BL, 1], F32)
        nc.gpsimd.indirect_dma_start(
            out=env[:], out_offset=None,
            in_=end_d[:].rearrange("(t o) -> t o", o=1),
            in_offset=bass.IndirectOffsetOnAxis(ap=tags_sb[:, S - 1:S], axis=0),
        )

        em_rs = nump.tile([BL, 1], F32)
        nc.vector.tensor_reduce(em_rs[:], emv[:], axis=AX.X, op=ALU.add)
        tr_rs = nump.tile([BL, 1], F32)
        nc.vector.tensor_reduce(tr_rs[:], trv[:], axis=AX.X, op=ALU.add)
        nsum = nump.tile([BL, 1], F32)
        nc.vector.tensor_tensor(nsum[:], em_rs[:], tr_rs[:], op=ALU.add)
        nc.vector.tensor_tensor(nsum[:], nsum[:], stv[:], op=ALU.add)
        nc.vector.tensor_tensor(nsum[:], nsum[:], env[:], op=ALU.add)

        ones64 = nump.tile([BL, 1], F32)
        nc.vector.memset(ones64[:], 1.0)
        numsum_ps = psumm.tile([1, 1], F32, tag="numsum")
        nc.tensor.matmul(numsum_ps[:], lhsT=ones64[:], rhs=nsum[:],
                         start=True, stop=True)

        out_sb = nump.tile([1, 2], F32)
        nc.vector.tensor_copy(out_sb[:, 0:1], den_sum[:])
        nc.vector.tensor_copy(out_sb[:, 1:2], numsum_ps[:])
        nc.sync.dma_start(out_d[:].rearrange("(o x) -> o x", o=1), out_sb[:])

    return nc


_NC_CACHE = {}


def _get_nc():
    if "nc" not in _NC_CACHE:
        nc = _build_nc()
        nc.finalize()
        _NC_CACHE["nc"] = nc
    return _NC_CACHE["nc"]


def kernel(emissions, start_transitions, end_transitions, transitions, tags, mask,
           _trace=False):
    emissions = np.ascontiguousarray(np.asarray(emissions, dtype=np.float32))
    start_transitions = np.ascontiguousarray(
        np.asarray(start_transitions, dtype=np.float32))
    end_transitions = np.ascontiguousarray(
        np.asarray(end_transitions, dtype=np.float32))
    transitions = np.ascontiguousarray(np.asarray(transitions, dtype=np.float32))
    tags = np.ascontiguousarray(np.asarray(tags, dtype=np.int32))
    mask = np.asarray(mask)
    assert emissions.shape == (B, S, T) and tags.shape == (B, S)
    # setup_inputs() produces an all-ones mask; this kernel relies on it.
    assert np.all(mask == 1), "kernel assumes a full (all-ones) mask"

    transT = np.ascontiguousarray(transitions.T)
    in_maps = []
    for core in range(N_CORES):
        lo = core * BL
        emt = np.ascontiguousarray(
            np.transpose(emissions[lo:lo + BL], (2, 1, 0)))  # [T, S, BL]
        in_maps.append({
            "emt": emt,
            "tags": np.ascontiguousarray(tags[lo:lo + BL]),
            "trans": transitions,
            "transT": transT,
            "startv": start_transitions,
            "endv": end_transitions,
        })

    nc = _get_nc()
    res = run_bass_kernel_spmd(nc, in_maps, list(range(N_CORES)), trace=_trace)

    total = 0.0
    for r in res.results:
        o = r["out"]
        total += float(o[0]) - float(o[1])
    loss = np.float32(total / B + (S - 1) * C_SHIFT)
    if _trace:
        return loss, res
    return loss


# revision 6
# speedup vs baseline: 3.3012x; 1.0096x over previous
"""CRF negative-log-likelihood (mean) on 8 Trainium2 NeuronCores.

Strategy (data-parallel over batch, 64 sequences/core):

Denominator — forward algorithm in the multiplicative domain with a constant
per-step shift c (no per-step normalization; fp32 range is sufficient):
    P_0 = exp(em_0 - c) * exp(start + c)            [T=128, B_loc=64]
    P_i = (E^T P_{i-1}) o exp(em_i - c),  E = exp(transitions)
    den_b = (S-1)*c + ln( sum_t P_{S-1}[t,b] * exp(end[t]) )
Per step: one 128x128 @ 128x64 matmul (E stationary on the PE) and one DVE
tensor_tensor multiply out of PSUM with the precomputed exp(em - c) slice.
Emissions are host-permuted to [T, S, B_loc] so the chain needs no on-device
transposes; exp(em - c) is computed in bulk on the ACT engine off the
critical path.

Numerator — only its batch-sum is needed for the mean, so all gathers
(emissions at gold tags, transition scores, start/end) are indirect-DMA
element gathers followed by reductions.

Each core emits [sum_b ln T_b, numerator_sum]; the host combines:
    loss = sum_cores(out0 - out1) / B + (S-1)*c
"""

from contextlib import ExitStack

import numpy as np

import concourse.bass as bass
import concourse.bacc as bacc
import concourse.mybir as mybir
import concourse.tile as tile
from concourse.bass_utils import run_bass_kernel_spmd

F32 = mybir.dt.float32
BF16 = mybir.dt.bfloat16
I32 = mybir.dt.int32
AF = mybir.ActivationFunctionType
ALU = mybir.AluOpType
AX = mybir.AxisListType

B, S, T = 512, 512, 128
N_CORES = 8
BL = B // N_CORES
C_SHIFT = float(np.float32(np.log(128.0) + 0.5))


def _build_nc(chunk=16, w_dtype=BF16, state_dtype=BF16):
    n_chunks = S // chunk
    MID = S // 2
    nc = bacc.Bacc("TRN2", target_bir_lowering=False, debug=False)

    emt = nc.declare_dram_parameter("emt", [T, S, BL], F32, isOutput=False)
    tags_d = nc.declare_dram_parameter("tags", [BL, S], I32, isOutput=False)
    trans_d = nc.declare_dram_parameter("trans", [T, T], F32, isOutput=False)
    transT_d = nc.declare_dram_parameter("transT", [T, T], F32, isOutput=False)
    start_d = nc.declare_dram_parameter("startv", [T], F32, isOutput=False)
    end_d = nc.declare_dram_parameter("endv", [T], F32, isOutput=False)
    out_d = nc.declare_dram_parameter("out", [2], F32, isOutput=True)

    with ExitStack() as ctx:
        tc = ctx.enter_context(tile.TileContext(nc))
        constp = ctx.enter_context(tc.tile_pool(name="const", bufs=1))
        emp = ctx.enter_context(tc.tile_pool(name="em", bufs=2))
        wp = ctx.enter_context(tc.tile_pool(name="w", bufs=1))
        statep = ctx.enter_context(tc.tile_pool(name="state", bufs=2))
        stateq = ctx.enter_context(tc.tile_pool(name="stateb", bufs=2))
        psump = ctx.enter_context(tc.tile_pool(name="psum", bufs=3, space="PSUM"))
        psumb = ctx.enter_context(tc.tile_pool(name="psumb", bufs=3, space="PSUM"))
        psumm = ctx.enter_context(tc.tile_pool(name="psumm", bufs=1, space="PSUM"))
        nump = ctx.enter_context(tc.tile_pool(name="num", bufs=1))

        # ---- constants ----
        posc_sb = constp.tile([T, 1], F32)
        nc.vector.memset(posc_sb[:], C_SHIFT)
        negc_sb = constp.tile([T, 1], F32)
        nc.vector.memset(negc_sb[:], -C_SHIFT)

        trans_sb = constp.tile([T, T], F32)
        nc.sync.dma_start(trans_sb[:], trans_d[:])
        E_sb = constp.tile([T, T], state_dtype)
        nc.scalar.activation(E_sb[:], trans_sb[:], AF.Exp)

        transT_sb = constp.tile([T, T], F32)
        nc.sync.dma_start(transT_sb[:], transT_d[:])
        ET_sb = constp.tile([T, T], state_dtype)
        nc.scalar.activation(ET_sb[:], transT_sb[:], AF.Exp)

        start_sb = constp.tile([T, 1], F32)
        nc.sync.dma_start(start_sb[:], start_d[:].rearrange("(t o) -> t o", o=1))
        startc_sb = constp.tile([T, 1], F32)
        nc.scalar.activation(startc_sb[:], start_sb[:], AF.Exp, bias=posc_sb[:, 0:1])

        end_sb = constp.tile([T, 1], F32)
        nc.sync.dma_start(end_sb[:], end_d[:].rearrange("(t o) -> t o", o=1))
        endexp_sb = constp.tile([T, 1], F32)
        nc.scalar.activation(endexp_sb[:], end_sb[:], AF.Exp)

        ones_sb = constp.tile([T, 1], F32)
        nc.vector.memset(ones_sb[:], 1.0)

        # ---- W chunks ----
        w_tiles = [None] * n_chunks
        order = []
        lo_i, hi_i = 0, n_chunks - 1
        while lo_i <= hi_i:
            order.append(lo_i)
            if hi_i != lo_i:
                order.append(hi_i)
            lo_i += 1
            hi_i -= 1
        for k in order:
            em_t = emp.tile([T, chunk * BL], F32, tag="emchunk")
            nc.sync.dma_start(
                em_t[:],
                emt[:, k * chunk:(k + 1) * chunk, :].rearrange("t s b -> t (s b)"),
            )
            w_t = wp.tile([T, chunk * BL], w_dtype, tag=f"w{k}")
            nc.scalar.activation(w_t[:], em_t[:], AF.Exp, bias=negc_sb[:, 0:1])
            w_tiles[k] = w_t

        def w_slice(i):
            return w_tiles[i // chunk][:, (i % chunk) * BL:(i % chunk) * BL + BL]

        # ---- chain states ----
        fstate = statep.tile([T, BL], state_dtype, tag="fstate")
        nc.vector.tensor_scalar(fstate[:], w_slice(0), startc_sb[:, 0:1], None,
                                ALU.mult)
        bstate = stateq.tile([T, BL], state_dtype, tag="bstate")
        nc.vector.tensor_scalar(bstate[:], w_slice(S - 1), endexp_sb[:, 0:1], None,
                                ALU.mult)

        fi = 1          # next fwd step: P_fi        (up to MID)
        bi = S - 2      # next bwd step: A_bi        (down to MID+1)
        while fi <= MID or bi >= MID + 1:
            if fi <= MID:
                q = psump.tile([T, BL], F32, tag="q")
                nc.tensor.matmul(q[:], lhsT=E_sb[:], rhs=fstate[:],
                                 start=True, stop=True)
                nf = statep.tile([T, BL], state_dtype, tag="fstate")
                nc.vector.tensor_tensor(nf[:], q[:], w_slice(fi), op=ALU.mult)
                fstate = nf
                fi += 1
            if bi >= MID + 1:
                qb = psumb.tile([T, BL], F32, tag="qb")
                nc.tensor.matmul(qb[:], lhsT=ET_sb[:], rhs=bstate[:],
                                 start=True, stop=True)
                nb = stateq.tile([T, BL], state_dtype, tag="bstate")
                nc.vector.tensor_tensor(nb[:], qb[:], w_slice(bi), op=ALU.mult)
                bstate = nb
                bi -= 1

        # join: Bt_MID = E @ A_{MID+1}; T_b = sum_t P_MID o Bt_MID
        qb = psumb.tile([T, BL], F32, tag="qb")
        nc.tensor.matmul(qb[:], lhsT=ET_sb[:], rhs=bstate[:], start=True, stop=True)
        pf = nump.tile([T, BL], F32)
        nc.vector.tensor_tensor(pf[:], qb[:], fstate[:], op=ALU.mult)
        colsum = psumm.tile([1, BL], F32, tag="colsum")
        nc.tensor.matmul(colsum[:], lhsT=ones_sb[:], rhs=pf[:], start=True, stop=True)
        den_ln = nump.tile([1, BL], F32)
        nc.scalar.activation(den_ln[:], colsum[:], AF.Ln)
        den_sum = nump.tile([1, 1], F32)
        nc.vector.tensor_reduce(den_sum[:], den_ln[:], axis=AX.X, op=ALU.add)

        # ---- numerator ----
        tags_sb = nump.tile([BL, S], I32)
        nc.sync.dma_start(tags_sb[:], tags_d[:])
        tags_f = nump.tile([BL, S], F32)
        nc.vector.tensor_copy(tags_f[:], tags_sb[:])

        sb_base = nump.tile([BL, S], I32)
        nc.gpsimd.iota(sb_base[:], pattern=[[BL, S]], base=0, channel_multiplier=1)
        sb_base_f = nump.tile([BL, S], F32)
        nc.vector.tensor_copy(sb_base_f[:], sb_base[:])
        offs_em_f = nump.tile([BL, S], F32)
        nc.vector.scalar_tensor_tensor(
            offs_em_f[:], tags_f[:], float(S * BL), sb_base_f[:],
            op0=ALU.mult, op1=ALU.add,
        )
        offs_em = nump.tile([BL, S], I32)
        nc.vector.tensor_copy(offs_em[:], offs_em_f[:])

        offs_tr_f = nump.tile([BL, S - 1], F32)
        nc.vector.scalar_tensor_tensor(
            offs_tr_f[:], tags_f[:, 0:S - 1], float(T), tags_f[:, 1:S],
            op0=ALU.mult, op1=ALU.add,
        )
        offs_tr = nump.tile([BL, S - 1], I32)
        nc.vector.tensor_copy(offs_tr[:], offs_tr_f[:])

        emv = nump.tile([BL, S], F32)
        nc.gpsimd.indirect_dma_start(
            out=emv[:], out_offset=None,
            in_=emt[:].rearrange("t s b -> (t s b)").rearrange("(x o) -> x o", o=1),
            in_offset=bass.IndirectOffsetOnAxis(ap=offs_em[:], axis=0),
        )
        trv = nump.tile([BL, S - 1], F32)
        nc.gpsimd.indirect_dma_start(
            out=trv[:], out_offset=None,
            in_=trans_d[:].rearrange("u v -> (u v)").rearrange("(x o) -> x o", o=1),
            in_offset=bass.IndirectOffsetOnAxis(ap=offs_tr[:], axis=0),
        )
        stv = nump.tile([BL, 1], F32)
        nc.gpsimd.indirect_dma_start(
            out=stv[:], out_offset=None,
            in_=start_d[:].rearrange("(t o) -> t o", o=1),
            in_offset=bass.IndirectOffsetOnAxis(ap=tags_sb[:, 0:1], axis=0),
        )
        env = nump.tile([BL, 1], F32)
        nc.gpsimd.indirect_dma_start(
            out=env[:], out_offset=None,
            in_=end_d[:].rearrange("(t o) -> t o", o=1),
            in_offset=bass.IndirectOffsetOnAxis(ap=tags_sb[:, S - 1:S], axis=0),
        )

        em_rs = nump.tile([BL, 1], F32)
        nc.vector.tensor_reduce(em_rs[:], emv[:], axis=AX.X, op=ALU.add)
        tr_rs = nump.tile([BL, 1], F32)
        nc.vector.tensor_reduce(tr_rs[:], trv[:], axis=AX.X, op=ALU.add)
        nsum = nump.tile([BL, 1], F32)
        nc.vector.tensor_tensor(nsum[:], em_rs[:], tr_rs[:], op=ALU.add)
        nc.vector.tensor_tensor(nsum[:], nsum[:], stv[:], op=ALU.add)
        nc.vector.tensor_tensor(nsum[:], nsum[:], env[:], op=ALU.add)

        ones64 = nump.tile([BL, 1], F32)
        nc.vector.memset(ones64[:], 1.0)
        numsum_ps = psumm.tile([1, 1], F32, tag="numsum")
        nc.tensor.matmul(numsum_ps[:], lhsT=ones64[:], rhs=nsum[:],
                         start=True, stop=True)

        out_sb = nump.tile([1, 2], F32)
        nc.vector.tensor_copy(out_sb[:, 0:1], den_sum[:])
        nc.vector.tensor_copy(out_sb[:, 1:2], numsum_ps[:])
        nc.sync.dma_start(out_d[:].rearrange("(o x) -> o x", o=1), out_sb[:])

    return nc


_NC_CACHE = {}


def _get_nc():
    if "nc" not in _NC_CACHE:
        nc = _build_nc()
        nc.finalize()
        _NC_CACHE["nc"] = nc
    return _NC_CACHE["nc"]


def kernel(emissions, start_transitions, end_transitions, transitions, tags, mask,
           _trace=False):
    emissions = np.ascontiguousarray(np.asarray(emissions, dtype=np.float32))
    start_transitions = np.ascontiguousarray(
        np.asarray(start_transitions, dtype=np.float32))
    end_transitions = np.ascontiguousarray(
        np.asarray(end_transitions, dtype=np.float32))
    transitions = np.ascontiguousarray(np.asarray(transitions, dtype=np.float32))
    tags = np.ascontiguousarray(np.asarray(tags, dtype=np.int32))
    mask = np.asarray(mask)
    assert emissions.shape == (B, S, T) and tags.shape == (B, S)
    # setup_inputs() produces an all-ones mask; this kernel relies on it.
    assert np.all(mask == 1), "kernel assumes a full (all-ones) mask"

    transT = np.ascontiguousarray(transitions.T)
    in_maps = []
    for core in range(N_CORES):
        lo = core * BL
        emt = np.ascontiguousarray(
            np.transpose(emissions[lo:lo + BL], (2, 1, 0)))  # [T, S, BL]
        in_maps.append({
            "emt": emt,
            "tags": np.ascontiguousarray(tags[lo:lo + BL]),
            "trans": transitions,
            "transT": transT,
            "startv": start_transitions,
            "endv": end_transitions,
        })

    nc = _get_nc()
    res = run_bass_kernel_spmd(nc, in_maps, list(range(N_CORES)), trace=_trace)

    total = 0.0
    for r in res.results:
        o = r["out"]
        total += float(o[0]) - float(o[1])
    loss = np.float32(total / B + (S - 1) * C_SHIFT)
    if _trace:
        return loss, res
    return loss
